# revision 1
# baseline (speedup 1.0000x reference)
"""DCT Frequency Splitter — Trainium2 Bass kernel.

Math: FFT2 -> mask -> IFFT2 -> real is a linear operator on the 196 patch
tokens (per channel).  low_sp = A @ patches with A = Re(Finv diag(m) F)
(196x196, real, built on host from the 4 mask params).  Since
high_mask = 1 - mask(high_params):  high_sp = patches - C @ patches with
C = A when low/high params coincide (the common case; then one matmul
feeds both outputs, and hi = x - lo_unscaled is a vector sub).  The token
mean feeding the gate MLP comes from tiny matmuls against a 1/196 column
(x block stationary), so the gate depends only on the loads.

Sharding: pure data parallel, batch 128 -> 16 per core across 8 cores.

The kernel is I/O bound (29 MB/core vs ~360 GB/s => ~81 us floor), so the
structure keeps the DMA engines dense (cost-model driven):
- main matmuls run as float32r (1 cycle/row at >=256 free cols vs 4 for
  fp32); the BIR verifier requires the producers of f32r-matmul inputs to
  write f32r-typed APs, so the x/weight load DMAs bitcast both sides.
- loads/stores are merged across image groups (every DMACopy holds the
  single HWDGE descriptor unit ~625ns: 100 DMAs = 62us of serialization),
  loads are issued one group ahead of compute, and stores one group late,
  so a store waiting on its scales never head-of-line-blocks a load.
- the gate MLP runs in fp8e4m3 off a small weight blob loaded first;
  its scales are ready before the first PSUM evacuation, which folds the
  lo scale into the evacuating activation (hi scales on DVE post-sub).
- sigmoid act table is preloaded at t=0; group sizes taper at the edges
  to shorten the first-store and last-store dependency tails.
- the single HWDGE descriptor unit issues one DMA per ~625ns, which
  bounds the kernel head: consts ship as two big blobs (low-precision
  gate weights in their own DMA — an f32r-typed DMA rounds its payload,
  so packed bits must never ride in one), pure-constant rows (alpha
  replication) are memset on-chip, and the all-zero b1 bias matmul is
  specialized away host-side.
- the CLS token row is never loaded (its operator column is zero): the
  contraction runs K=127+69 over patch tokens only, and the operator
  ships without the dead mean row so output rows align with input
  partitions.  CLS passes through via two HBM->HBM DMAs.  The DMA
  engines then run back-to-back from first load to last store.
"""

import os
import numpy as np

import concourse.bass as bass
import concourse.bacc as bacc_mod
import concourse.mybir as mybir
import concourse.tile as tile
from concourse.bass_utils import run_bass_kernel_spmd

H, W = 14, 14
B, N, D = 128, 197, 768
P = 196  # patch tokens
NCORES = 8
BS = B // NCORES  # batches per core

# tunables (env overridable for experiments)
GRP = int(os.environ.get("KRN_GROUP", "4"))       # images per DMA/gate group
MM_DT = os.environ.get("KRN_MM_DT", "f32r")        # f32 | f32r
BUFX = int(os.environ.get("KRN_BUFX", "3"))        # x-tile generations in flight
BUFO = int(os.environ.get("KRN_BUFO", "2"))        # out-tile generations
F32 = mybir.dt.float32
F32R = mybir.dt.float32r
BF16 = mybir.dt.bfloat16
FP8 = mybir.dt.float8e4


def _freq_mask_np(params, low):
    ch, cw, radius, sharp = [np.float64(v) for v in np.asarray(params)]
    y = np.arange(H, dtype=np.float64)
    x = np.arange(W, dtype=np.float64)
    d2 = (y[:, None] - ch) ** 2 + (x[None, :] - cw) ** 2
    dist = np.sqrt(d2 + 1e-12)
    s = np.clip(sharp, 0.5, 10.0)
    r = np.clip(radius, 1.0, min(H, W) / 2.0)
    m = np.exp(-((dist / r) ** s))
    return m if low else 1.0 - m


def _conv_operator(mask):
    """Real 196x196 operator equivalent to ifft2(fft2(img)*mask).real."""
    F_H = np.exp(-2j * np.pi * np.outer(np.arange(H), np.arange(H)) / H)
    F_W = np.exp(-2j * np.pi * np.outer(np.arange(W), np.arange(W)) / W)
    Fi_H = np.conj(F_H) / H
    Fi_W = np.conj(F_W) / W
    op = np.kron(Fi_H, Fi_W) @ np.diag(mask.ravel()) @ np.kron(F_H, F_W)
    return np.real(op)


def _mm_ap(ap):
    if MM_DT == "f32r":
        return ap.bitcast(F32R)
    return ap


def _ld_ap(ap):
    # f32r-typed view for DMA producer/consumer APs feeding f32r matmuls
    if MM_DT == "f32r":
        return ap.bitcast(F32R)
    return ap


def _build_program(consts, share_Y, b2lo, b2hi, alo, ahi):
    nc = bacc_mod.Bacc(None)

    xs_h = nc.dram_tensor("xs", [BS, N, D], F32, kind="ExternalInput")
    lo_h = nc.dram_tensor("lo", [BS, N, D], F32, kind="ExternalOutput")
    hi_h = nc.dram_tensor("hi", [BS, N, D], F32, kind="ExternalOutput")

    ch = {k: nc.inline_tensor(v, name=f"c_{k}") for k, v in consts.items()}

    Copy = mybir.ActivationFunctionType.Copy
    Sig = mybir.ActivationFunctionType.Sigmoid

    NSPLIT = [(0, 512), (512, 768)]

    with tile.TileContext(nc) as tc:
        with (
            tc.tile_pool(name="consts", bufs=1) as cp,
            tc.tile_pool(name="xp", bufs=BUFX) as xp,
            tc.tile_pool(name="outp", bufs=BUFO) as outp,
            tc.tile_pool(name="gp", bufs=2) as gp,
            tc.tile_pool(name="pm", bufs=2, space="PSUM") as pm,
            tc.tile_pool(name="par", bufs=2, space="PSUM") as par,
        ):
            # variable group sizes: small first group so the matmul pipe
            # starts early, small last group so the final gate+store tail
            # is short
            gs_env = os.environ.get("KRN_GS", "")
            if gs_env:
                group_sizes = [int(v) for v in gs_env.split(",")]
                assert sum(group_sizes) == BS and max(group_sizes) <= GRP
            elif GRP == 4 and BS == 16:
                group_sizes = [1, 4, 4, 4, 2, 1]
            else:
                group_sizes = [GRP] * (BS // GRP)
                if BS % GRP:
                    group_sizes.append(BS % GRP)

            # the first x load leads the queue: DMAs issue one per ~625ns
            # through the single HWDGE unit, and the const blobs (fp8 gate
            # weights + f32r operator) are each shorter than that cadence —
            # behind the 1.1us first-image transfer their issue latency
            # hides.  The fp8/bf16 gate weights need their OWN dma: an
            # f32r-typed DMA rounds its payload on hardware (that is what
            # the verifier rule is about), corrupting bit-packed data.
            # tokens pack two per partition ((p two) d -> p (two d)): one
            # load DMA per image/group, no 69-row half, no dead weight rows.
            # The CLS token is never loaded (its operator column is zero).
            D2 = 2 * D
            xa0 = xp.tile([98, GRP * D2], F32, tag="xa")
            nc.sync.dma_start(
                out=_ld_ap(xa0[:, 0:D2]),
                in_=_ld_ap(xs_h[0, 1:197, :].rearrange(
                    "(p two) d -> p (two d)", two=2)))
            gc = cp.tile([128, 1156], FP8, tag="gcrit")
            nc.sync.dma_start(out=gc[:], in_=ch["gcrit"][...])
            wa = cp.tile([98, 398], F32, tag="wtblob")
            nc.sync.dma_start(out=_ld_ap(wa[0:98, :]),
                              in_=_ld_ap(ch["wtblob"][...]))
            for j in range(1, group_sizes[0]):
                s = j * D2
                nc.sync.dma_start(
                    out=_ld_ap(xa0[:, s:s + D2]),
                    in_=_ld_ap(xs_h[j, 1:197, :].rearrange(
                        "(p two) d -> p (two d)", two=2)))
            # W_xy: in-half x, out-half y, each [98, 98]
            wt_ee = wa[0:98, 0:98]
            wt_oe = wa[0:98, 98:196]
            wt_eo = wa[0:98, 196:294]
            wt_oo = wa[0:98, 294:392]
            mc = wa[0:98, 392:393]      # token-mean weights
            ident = wa[0:4, 393:397]
            gate_consts = {
                "w1c": gc[:, 0:1152].rearrange("p (a b) -> p a b", a=6),
                "w2c0": gc[:, 1152:1154],     # [128, 2]
                "w2c1": gc[0:64, 1154:1156],  # [64, 2]
            }
            # preload the sigmoid activation table during the first loads
            # (the table load costs ~1.3us and would otherwise land on the
            # first gate's critical path)
            warm = gp.tile([1, 1], F32, tag="warm")
            nc.scalar.activation(warm[:], wa[0:1, 393:394], Sig)

            def load_deferred():
                # CLS passthrough for all batches in two strided DMAs
                nc.sync.dma_start(out=lo_h[:, 0:1, :], in_=xs_h[:, 0:1, :])
                nc.sync.dma_start(out=hi_h[:, 0:1, :], in_=xs_h[:, 0:1, :])

            def gate_mlp(Gn, arena, xa):
                """Gate MLP for a group, computed straight from the x tile
                (means via tiny matmuls with the x block stationary and a
                1/196 column moving; the two packed token halves accumulate).
                Depends only on the loads, so it runs concurrently with the
                group's main matmuls and its scales are ready by the time
                the first evacuation needs them."""
                for j in range(Gn):
                    s = j * D2
                    for c in range(6):
                        # plain fp32: free size 1 violates the fp32r ISA
                        # restrictions, and 4 cycles/row is free at this size
                        col = arena[:, c * 16 + j:c * 16 + j + 1]
                        nc.tensor.matmul(col, xa[:, s + c * 128:
                                                 s + (c + 1) * 128],
                                         mc, start=True, stop=False)
                        nc.tensor.matmul(col, xa[:, s + D + c * 128:
                                                 s + D + (c + 1) * 128],
                                         mc, start=False, stop=True)
                gT = gp.tile([128, 6, 16], FP8, tag="gT")
                nc.vector.tensor_copy(
                    gT[:].rearrange("p a b -> p (a b)"), arena[:, 0:96])

                h_ps = arena[0:16, 96:288]
                has_b1 = "b1c" in gate_consts
                for c in range(6):
                    nc.tensor.matmul(h_ps[0:Gn, :], gT[:, c, 0:Gn],
                                     gate_consts["w1c"][:, c, :],
                                     start=(c == 0),
                                     stop=(not has_b1 and c == 5))
                if "b1c" in gate_consts:
                    nc.tensor.matmul(h_ps[0:Gn, :],
                                     gate_consts["ones1"][0:1, 0:Gn],
                                     gate_consts["b1c"][0:1, :], start=False,
                                     stop=True)
                hs = gp.tile([16, 192], F32, tag="hs")
                nc.vector.tensor_relu(hs[0:Gn, :], h_ps[0:Gn, :])

                hT = gp.tile([128, 2, 16], FP8, tag="hT")
                nc.tensor.transpose(arena[:, 288:288 + Gn], hs[0:Gn, 0:128],
                                    ident[0:Gn, 0:Gn])
                nc.tensor.transpose(arena[0:64, 304:304 + Gn],
                                    hs[0:Gn, 128:192], ident[0:Gn, 0:Gn])
                nc.vector.tensor_copy(hT[:].rearrange("p a b -> p (a b)"),
                                      arena[:, 288:320])

                # final layer: two M=1 matmuls (gate rows at partition 0);
                # b2 folds into the sigmoid bias, alpha into a post-scale;
                # rows then replicated across partitions via K=1 matmuls
                crows = []
                for col, b2f in ((0, b2lo), (1, b2hi)):
                    g_ps = arena[0:1, 320 + 16 * col:336 + 16 * col]
                    nc.tensor.matmul(g_ps[:, 0:Gn],
                                     gate_consts["w2c0"][:, col:col + 1],
                                     hT[:, 0, 0:Gn], start=True, stop=False)
                    nc.tensor.matmul(g_ps[:, 0:Gn],
                                     gate_consts["w2c1"][:, col:col + 1],
                                     hT[0:64, 1, 0:Gn], start=False, stop=True)
                    cr = gp.tile([1, 16], F32, tag=f"crow{col}")
                    nc.scalar.activation(cr[:, 0:Gn], g_ps[:, 0:Gn], Sig,
                                         bias=b2f)
                    crows.append(cr)
                # replication matmuls against alpha-scaled ones rows fold the
                # alpha multiply in; one copy lands both gate vectors
                for col, wrow in ((0, "alr"), (1, "ahr")):
                    nc.tensor.matmul(
                        arena[:, 352 + 16 * col:352 + 16 * col + Gn],
                        gate_consts[wrow][0:1, :],
                        crows[col][0:1, 0:Gn],
                        start=True, stop=True)
                crlh = gp.tile([128, 32], F32, tag="crlh")
                nc.vector.tensor_copy(crlh[:], arena[:, 352:384])
                return crlh[:, 0:16], crlh[:, 16:32]
            def flush_stores(b0, Gn, tiles):
                """Merged stores (one DMA per output tensor per group),
                traced TWO groups late (right after the next group's loads)
                so their semaphore waits never head-of-line-block load
                prefetch in the SP DMA queue."""
                lo_ga, hi_ga = tiles
                # 2-image sub-blocks: each store becomes eligible as soon
                # as its own images' scales land, instead of the whole group
                for c0 in range(0, Gn, 2):
                    c1 = min(c0 + 2, Gn)
                    cn = c1 - c0
                    nc.sync.dma_start(
                        out=lo_h[b0 + c0:b0 + c1, 1:197, :].rearrange(
                            "g (p two) d -> p g (two d)", two=2),
                        in_=lo_ga[0:98, c0 * D2:c1 * D2].rearrange(
                            "p (g td) -> p g td", g=cn))
                    nc.sync.dma_start(
                        out=hi_h[b0 + c0:b0 + c1, 1:197, :].rearrange(
                            "g (p two) d -> p g (two d)", two=2),
                        in_=hi_ga[0:98, c0 * D2:c1 * D2].rearrange(
                            "p (g td) -> p g td", g=cn))

            def trace_loads(b0, Gn, per_image):
                """One merged load DMA per group (tokens packed two per
                partition; each descriptor is a 6KB two-token run)."""
                w = Gn * D2
                xa = xp.tile([98, GRP * D2], F32, tag="xa")
                if per_image:
                    for j in range(Gn):
                        s = j * D2
                        nc.sync.dma_start(
                            out=_ld_ap(xa[:, s:s + D2]),
                            in_=_ld_ap(xs_h[b0 + j, 1:197, :].rearrange(
                                "(p two) d -> p (two d)", two=2)))
                else:
                    for c0 in range(0, Gn, 2):
                        c1 = min(c0 + 2, Gn)
                        nc.sync.dma_start(
                            out=_ld_ap(xa[:, c0 * D2:c1 * D2].rearrange(
                                "p (g td) -> p g td", g=c1 - c0)),
                            in_=_ld_ap(
                                xs_h[b0 + c0:b0 + c1, 1:197, :].rearrange(
                                    "g (p two) d -> p g (two d)", two=2)))
                return (xa,)

            starts = [0]
            for Gn in group_sizes:
                starts.append(starts[-1] + Gn)

            pending_store = None   # group awaiting its output stores
            # loads run one group ahead of compute so they sit in the SP
            # queue ahead of older groups' store bursts: the DMA engines
            # always have eligible load work while a store waits on scales
            xts = {0: (xa0,)}
            if "grow" in ch:
                # generic path (b1 != 0): single-row gate constants (bias
                # row, ones row, alpha rows as bit-packed fp32), slotted
                # between L0 and L1 where the issue cadence has a free slot
                gr = cp.tile([1, 720], BF16, tag="grow")
                nc.sync.dma_start(out=gr[:], in_=ch["grow"][...])
                galr = gr[0:1, 208:720].bitcast(F32)   # [1, 256] fp32 view
                gate_consts["b1c"] = gr[0:1, 0:192]
                gate_consts["ones1"] = gr[0:1, 192:208]
                gate_consts["alr"] = galr[:, 0:128]
                gate_consts["ahr"] = galr[:, 128:256]
            else:
                # b1 == 0 (the reference): no bias matmul, and the alpha
                # replication rows are pure constants -> memset instead of a
                # DMA, freeing a head HWDGE cadence slot
                alr_t = cp.tile([1, 128], F32, tag="alr")
                nc.vector.memset(alr_t[:], alo)
                ahr_t = cp.tile([1, 128], F32, tag="ahr")
                nc.vector.memset(ahr_t[:], ahi)
                gate_consts["alr"] = alr_t[0:1, :]
                gate_consts["ahr"] = ahr_t[0:1, :]
            for g, Gn in enumerate(group_sizes):
                b0 = starts[g]
                w = Gn * D
                # per-group psum arena for the gate pipeline: 0:96 gT |
                # 96:288 h | 288:320 hT | 320:352 gate | 352:384 replication
                arena = par.tile([128, 512], F32, tag="arena")

                if g + 1 < len(group_sizes):
                    xts[g + 1] = trace_loads(starts[g + 1],
                                             group_sizes[g + 1], False)
                if g == 0:
                    load_deferred()
                if pending_store is not None:
                    flush_stores(*pending_store)
                    pending_store = None
                (xa,) = xts.pop(g)

                lo_ga = outp.tile([98, GRP * D2], F32, tag="lo_ga")
                hi_ga = outp.tile([98, GRP * D2], F32, tag="hi_ga")

                # gate first: independent of the mains, so its latency hides
                # behind them and the scales below never wait
                crl, crh = gate_mlp(Gn, arena, xa)

                for j in range(Gn):
                    s = j * D2
                    # Y = OP @ x[b] with tokens packed two per partition:
                    # out half y at cols y*768.., fed by both in halves.
                    # Evac/sub/scale run per half so the Act/DVE pipeline
                    # keeps its fine-grained interleave with the matmuls.
                    ylo = pm.tile([98, D2], F32, tag="ym")
                    for h, (wt_e, wt_o) in ((0, (wt_ee, wt_oe)),
                                            (1, (wt_eo, wt_oo))):
                        # chunk at PSUM bank boundaries (2KB each)
                        chunks = ([(0, 512), (512, 768)] if h == 0 else
                                  [(0, 256), (256, 768)])
                        o0 = h * D
                        for (n0, n1) in chunks:
                            nc.tensor.matmul(ylo[:, o0 + n0:o0 + n1],
                                             _mm_ap(wt_e),
                                             _mm_ap(xa[:, s + n0:s + n1]),
                                             start=True, stop=False)
                            nc.tensor.matmul(ylo[:, o0 + n0:o0 + n1],
                                             _mm_ap(wt_o),
                                             _mm_ap(xa[:, s + D + n0:
                                                       s + D + n1]),
                                             start=False, stop=True)
                        nc.scalar.activation(lo_ga[:, s + o0:s + o0 + D],
                                             ylo[:, o0:o0 + D], Copy,
                                             scale=crl[0:98, j:j + 1])
                        nc.vector.tensor_sub(hi_ga[:, s + o0:s + o0 + D],
                                             xa[:, s + o0:s + o0 + D],
                                             ylo[:, o0:o0 + D])
                        nc.vector.tensor_scalar_mul(
                            hi_ga[:, s + o0:s + o0 + D],
                            hi_ga[:, s + o0:s + o0 + D],
                            crh[0:98, j:j + 1])

                pending_store = (b0, Gn, (lo_ga, hi_ga))

            if pending_store is not None:
                flush_stores(*pending_store)
    if not nc.is_finalized():
        nc.finalize()
    return nc


def kernel(x, low_params, high_params, alpha_low, alpha_high,
           w1, b1, w2, b2, cls_token_idx):
    assert int(cls_token_idx) == 0
    x = np.ascontiguousarray(np.asarray(x, dtype=np.float32))
    assert x.shape == (B, N, D)

    lm = _freq_mask_np(low_params, True)
    A = _conv_operator(lm)                       # low operator [196, 196]
    share_Y = np.allclose(np.asarray(low_params, np.float32),
                          np.asarray(high_params, np.float32))
    Cm = A if share_Y else _conv_operator(_freq_mask_np(high_params, True))

    w1 = np.asarray(w1, np.float32)
    sig = lambda v: 1.0 / (1.0 + np.exp(-np.float64(v)))

    def make_consts(OP):
        # tokens pack two per partition (patch 2k/2k+1 -> partition k), so
        # both K-halves span the same 98 partitions and the operator ships
        # with zero dead rows: four [98,98] blocks by (in-half, out-half).
        # W_xy[k, m] = OP[out_half_y(m), in_half_x(k)]
        OPd = np.asarray(OP, np.float64)
        import ml_dtypes
        wtblob = np.zeros((98, 398), np.float32)
        for bi, (ih, oh) in enumerate(((0, 0), (1, 0), (0, 1), (1, 1))):
            # lhsT layout [K=in, M=out]
            wtblob[:, bi * 98:(bi + 1) * 98] = \
                OPd[oh::2, ih::2].T.astype(np.float32)
        wtblob[:, 392] = 1.0 / P       # token-mean weights (both halves)
        wtblob[0:4, 393:397] = np.eye(4, dtype=np.float32)
        gcrit = np.zeros((128, 1156), np.float32)
        gcrit[:, 0:1152] = w1.reshape(6, 128, 192).transpose(1, 0, 2).reshape(128, 1152)
        gcrit[:, 1152:1154] = np.asarray(w2, np.float32)[0:128]
        gcrit[0:64, 1154:1156] = np.asarray(w2, np.float32)[128:192]
        out = {"wtblob": wtblob,
               "gcrit": gcrit.astype(ml_dtypes.float8_e4m3)}
        if np.any(np.asarray(b1, np.float32)):
            grow = np.zeros((1, 720), np.float32)
            grow[0, 0:192] = np.asarray(b1, np.float32)
            grow[0, 192:208] = 1.0
            growb = grow.astype(ml_dtypes.bfloat16)
            galr = np.zeros((1, 256), np.float32)
            galr[0, 0:128] = sig(alpha_low)
            galr[0, 128:256] = sig(alpha_high)
            growb[0, 208:720] = galr.view(ml_dtypes.bfloat16)
            out["grow"] = growb
        return out

    b2v = np.asarray(b2, np.float64).reshape(2)

    def run_once(consts):
        nc = _build_program(consts, True,
                            b2lo=float(b2v[0]), b2hi=float(b2v[1]),
                            alo=float(sig(alpha_low)), ahi=float(sig(alpha_high)))
        xs = x.reshape(NCORES, BS, N, D)
        in_maps = [{"xs": np.ascontiguousarray(xs[c])} for c in range(NCORES)]
        want_trace = bool(int(os.environ.get("KRN_TRACE", "0")))
        try:
            res = run_bass_kernel_spmd(nc, in_maps, core_ids=list(range(NCORES)),
                                       trace=want_trace)
        except ModuleNotFoundError:
            res = run_bass_kernel_spmd(nc, in_maps, core_ids=list(range(NCORES)))
        lo = np.concatenate([r["lo"] for r in res.results], axis=0)
        hi = np.concatenate([r["hi"] for r in res.results], axis=0)
        if getattr(res, "exec_time_ns", None) is not None:
            print(f"HW exec time: {res.exec_time_ns} ns")
        return lo, hi

    if share_Y:
        return run_once(make_consts(A))
    # generic case (never hit by the reference inputs): two passes of the
    # validated single-operator program — lo from the A pass, hi from the C
    # pass (the gate depends only on x, so it is identical in both)
    lo, _ = run_once(make_consts(A))
    _, hi = run_once(make_consts(Cm))
    return lo, hi

